# revision 34
# baseline (speedup 1.0000x reference)
"""Trainium2 Bass kernel for nn_Attention_75814762709205.

Computation (per batch row b, seq s):
    proj  = hidden_enc @ W + b          [B,S,D]
    score = hidden_dec.T * proj         (per-channel scale)
    attn  = softmax(score, axis=-1)     (over D)
    out   = sum_s attn * hidden_enc     [B,D]

Sharding: data-parallel over batch, 4 batches per core on 8 cores.

Precision strategy (validated numerically against the exact reference
data): the softmax error from an fp8 matmul is amplified by exp() in
proportion to |dec_e|, so the channels are permuted by |dec| on the
host.  The top NB=256 channels go through a bf16 matmul; the remaining
NF=768 channels go through an fp8e4 DoubleRow matmul (2 k-chunks per
pass, ~1.8x the bf16 matmul rate).  dec is folded into W on the host
(score = enc @ (W*dec)), and the channel permutation is applied to both
W axes so a single permuted bf16 copy of enc serves as the matmul
contraction input and the attention values.  Softmax denominator is
folded into the final sequence reduction: context = sum_s r_s *
(exp_s * enc_s) with r = 1/sum(exp), computed as a matmul with r as the
stationary vector.

The transposed enc needed for the matmul stationaries is prepared on
the HOST (numpy reshape/transpose into the exact SBUF tile layout) and
DMA'd as a second stream, so the PE does no transposes at all; the fp8
copy of the transposed tile is converted on DVE.  The ctx matmuls run
two tiles behind so the PE never waits on the ACT/DVE softmax tail.
"""

import sys

sys.path.insert(0, "/opt/trn_rl_repo")

from collections import deque

import numpy as np
import ml_dtypes

import concourse.bass as bass
import concourse.mybir as mybir
import concourse.tile as tile
from concourse import bacc, bass_utils

B, S, D = 32, 2048, 1024
NCORES = 8
BPC = B // NCORES  # batches per core
ROWS = BPC * S  # rows per core
P = 128
NT = ROWS // P  # row tiles per core
TPB = S // P  # row tiles per batch
KC = D // P  # contraction chunks
NB = 256  # bf16 (high |dec|) channels
NF = D - NB  # fp8 channels
W8SCALE = 8.0  # fp8 W pre-scale (undone in the exp activation)

F32 = mybir.dt.float32
BF16 = mybir.dt.bfloat16
F8 = mybir.dt.float8e4
AF = mybir.ActivationFunctionType
DR = mybir.MatmulPerfMode.DoubleRow
DRS = mybir.MatmulPerfMode.DoubleRowSwInterleave

NP_BF16 = ml_dtypes.bfloat16
NP_F8 = ml_dtypes.float8_e4m3


def build_program(with_bias: bool, repeats: int = 1):
    nc = bacc.Bacc("TRN2", target_bir_lowering=False, debug=False)
    enc_d = nc.dram_tensor("enc", [ROWS, D], BF16, kind="ExternalInput")
    encT_d = nc.dram_tensor("encT", [NT, P, KC, P], BF16, kind="ExternalInput")
    encT8_d = nc.dram_tensor("encT8", [NT, P, KC // 2, 2 * P], F8, kind="ExternalInput")
    wb_d = nc.dram_tensor("wb", [P, KC, NB], BF16, kind="ExternalInput")
    w8_d = nc.dram_tensor("w8", [P, KC, NF], F8, kind="ExternalInput")
    bb_d = b8_d = None
    if with_bias:
        bb_d = nc.dram_tensor("bb", [1, NB], BF16, kind="ExternalInput")
        b8_d = nc.dram_tensor("b8", [1, NF], BF16, kind="ExternalInput")
    out_d = nc.dram_tensor("out", [BPC, D], F32, kind="ExternalOutput")

    NTOT = repeats * NT

    with tile.TileContext(nc) as tc:
        with (
            tc.tile_pool(name="consts", bufs=1) as consts,
            tc.tile_pool(name="wpool", bufs=1) as wpool,
            tc.tile_pool(name="encp", bufs=6) as encp,
            tc.tile_pool(name="encTp", bufs=5) as encTp,
            tc.tile_pool(name="encT8p", bufs=5) as encT8p,
            tc.tile_pool(name="expp", bufs=3) as expp,
            tc.tile_pool(name="wtp", bufs=4) as wtp,
            tc.tile_pool(name="smalls", bufs=12) as smalls,
            tc.tile_pool(name="outp", bufs=2) as outp,
            tc.tile_pool(name="pr_ps", bufs=3, space=bass.MemorySpace.PSUM) as pr_ps,
            tc.tile_pool(name="ctx_ps", bufs=1, space=bass.MemorySpace.PSUM) as ctx_ps,
        ):
            # ---- pipeline stages ----
            # three DMA streams per tile, issue spread over SP and ScalarE
            # queues (ScalarE is also a HWDGE engine) to avoid serializing on
            # one descriptor queue
            loaded = {}

            def load(t):
                enc_t = encp.tile([P, D], BF16, name="enc_t")
                nc.sync.dma_start(
                    enc_t, enc_d.ap()[(t % NT) * P : (t % NT + 1) * P, :]
                )
                encT = encTp.tile([P, KC, P], BF16, name="encT")
                nc.gpsimd.dma_start(encT, encT_d.ap()[t % NT])
                encT8 = encT8p.tile([P, KC // 2, 2 * P], F8, name="encT8")
                nc.scalar.dma_start(encT8, encT8_d.ap()[t % NT])
                loaded[t] = (enc_t, encT, encT8)

            state = {"ctx_half": None}

            def emit_ctx(prev):
                ones_col, wt, tib, bidx = prev
                if tib == 0:
                    state["ctx_half"] = [
                        ctx_ps.tile([1, 512], F32, name=f"ctxh{h}") for h in range(2)
                    ]
                for h2 in range(2):
                    nc.tensor.matmul(
                        state["ctx_half"][h2],
                        ones_col,
                        wt[:, h2 * 512 : (h2 + 1) * 512],
                        start=(tib == 0),
                        stop=(tib == TPB - 1),
                    )
                if tib == TPB - 1:
                    ctx_sb = outp.tile([1, D], F32, name="ctx_sb")
                    for h2 in range(2):
                        nc.vector.tensor_copy(
                            ctx_sb[:, h2 * 512 : (h2 + 1) * 512],
                            state["ctx_half"][h2],
                        )
                    nc.sync.dma_start(out_d.ap()[bidx : bidx + 1, :], ctx_sb)

            # bootstrap the enc pipeline BEFORE the W setup DMAs
            load(0)
            load(1)
            load(2)

            wb_sb = wpool.tile([P, KC, NB], BF16)
            nc.sync.dma_start(wb_sb, wb_d.ap())
            w8_sb = wpool.tile([P, KC, NF], F8)
            nc.sync.dma_start(w8_sb, w8_d.ap())

            # constant ones stationary for the ctx (sequence-sum) matmuls;
            # the softmax reciprocal is folded into wt on DVE instead
            ones_f = consts.tile([P, 1], F32)
            nc.any.memset(ones_f, 1.0)
            ones_col = consts.tile([P, 1], BF16)
            nc.vector.tensor_copy(ones_col, ones_f)

            ones_row = None
            bb_sb = b8_sb = None
            if with_bias:
                ones_f32 = consts.tile([1, P], F32)
                nc.any.memset(ones_f32, 1.0)
                ones_row = consts.tile([1, P], BF16)
                nc.vector.tensor_copy(ones_row, ones_f32)
                bb_sb = consts.tile([1, NB], BF16)
                nc.sync.dma_start(bb_sb, bb_d.ap())
                b8_sb = consts.tile([1, NF], BF16)
                nc.sync.dma_start(b8_sb, b8_d.ap())

            pending = deque()
            for t in range(NTOT):
                bidx, tib = divmod(t % NT, TPB)
                if t + 3 < NTOT:
                    load(t + 3)
                enc_t, encT, encT8 = loaded.pop(t)

                # score layout (permuted channels): cols 0:NB bf16 part,
                # NB:1024 fp8 part.  One 2-bank PSUM tile; bank0 holds
                # [bf16 256 | fp8 256], bank1 holds fp8 512.
                pr = pr_ps.tile([P, 1024], F32, name="pr")

                # bf16 part: shares bank0 with the first fp8 region
                # (disjoint columns; the fp8 matmuls use start=False and
                # overwrite-on-first-touch via has_written)
                for k in range(KC):
                    nc.tensor.matmul(
                        pr[:, 0:NB],
                        encT[:, k, :],
                        wb_sb[:, k, :],
                        start=(k == 0),
                        stop=False,
                        skip_group_check=True,
                    )
                # fp8 DoubleRow part (software-interleaved stationary: the
                # host lays each k-pair out contiguously, so LDWEIGHTS is a
                # linear read instead of the HW interleave pattern)
                for c in range(KC // 2):
                    if c == 2 and len(pending) >= 2:
                        emit_ctx(pending.popleft())
                    ks = slice(2 * c, 2 * c + 2)
                    last = c == KC // 2 - 1
                    nc.tensor.matmul(
                        pr[:, NB:512],
                        encT8[:, c, :],
                        w8_sb[:, ks, 0 : 512 - NB],
                        start=False,
                        stop=(last and not with_bias),
                        perf_mode=DRS,
                        skip_group_check=True,
                    )
                    nc.tensor.matmul(
                        pr[:, 512:1024],
                        encT8[:, c, :],
                        w8_sb[:, ks, 512 - NB : NF],
                        start=(c == 0),
                        stop=(last and not with_bias),
                        perf_mode=DRS,
                        skip_group_check=True,
                    )
                if with_bias:
                    nc.tensor.matmul(
                        pr[:, 0:NB],
                        ones_row,
                        bb_sb,
                        start=False,
                        stop=False,
                        skip_group_check=True,
                    )
                    nc.tensor.matmul(
                        pr[:, NB:512],
                        ones_row,
                        b8_sb[:, 0 : 512 - NB],
                        start=False,
                        stop=True,
                        skip_group_check=True,
                    )
                    nc.tensor.matmul(
                        pr[:, 512:1024],
                        ones_row,
                        b8_sb[:, 512 - NB : NF],
                        start=False,
                        stop=True,
                        skip_group_check=True,
                    )

                # single exp over both banks with fused row-sum; the whole W
                # carries the W8SCALE pre-scale (exact power of 2), undone here
                ssum = smalls.tile([P, 1], F32)
                exp_t = expp.tile([P, D], BF16)
                nc.scalar.activation(
                    exp_t, pr, AF.Exp, scale=1.0 / W8SCALE, accum_out=ssum
                )

                recip_f = smalls.tile([P, 1], F32)
                nc.vector.reciprocal(recip_f, ssum)

                # wt = (exp * 1/rowsum) * enc fused on DVE; ctx then reduces
                # over rows with a constant ones stationary
                wt = wtp.tile([P, D], BF16)
                nc.vector.scalar_tensor_tensor(
                    wt, exp_t, recip_f, enc_t,
                    op0=mybir.AluOpType.mult, op1=mybir.AluOpType.mult,
                )

                pending.append((ones_col, wt, tib, bidx))
            while pending:
                emit_ctx(pending.popleft())

    nc.compile()
    return nc


def _perm(dec):
    return np.argsort(-np.abs(dec), kind="stable")


def make_in_maps(hidden_dec, hidden_enc, W, b):
    enc = np.asarray(hidden_enc, dtype=np.float32).reshape(B, S, D)
    W = np.asarray(W, dtype=np.float32)
    dec = np.asarray(hidden_dec, dtype=np.float32).reshape(D)
    b = np.asarray(b, dtype=np.float32).reshape(D)
    with_bias = bool(np.any(b != 0.0))

    perm = _perm(dec)
    Weff = W * dec[None, :]
    Wp = Weff[np.ix_(perm, perm)]
    wb = np.ascontiguousarray(
        (Wp[:, :NB] * W8SCALE).reshape(KC, P, NB).transpose(1, 0, 2)
    ).astype(NP_BF16)
    w8 = np.ascontiguousarray(
        (Wp[:, NB:] * W8SCALE).reshape(KC, P, NF).transpose(1, 0, 2)
    ).astype(NP_F8)
    encp = enc[:, :, perm].astype(NP_BF16)

    b_eff = (b * dec)[perm] * W8SCALE
    bb = b_eff[:NB].reshape(1, NB).astype(NP_BF16)
    b8 = b_eff[NB:].reshape(1, NF).astype(NP_BF16)

    in_maps = []
    for c in range(NCORES):
        ev = encp[c * BPC : (c + 1) * BPC].reshape(ROWS, D)
        # host-side tiled transpose into the exact SBUF stationary layout:
        # encT[t, p, kc, r] = enc[t*128 + r, kc*128 + p]
        encT = np.ascontiguousarray(
            ev.reshape(NT, P, KC, P).transpose(0, 3, 2, 1)
        )
        # fp8 copy, software-interleaved for DoubleRowSwInterleave: per
        # partition each k-pair's stationary stream is
        # [A_col127, B_col127, A_col126, ..., B_col0] (A/B = the two
        # k-chunks, columns reversed)
        e8rev = encT.astype(NP_F8)[:, :, :, ::-1]
        enc8i = np.ascontiguousarray(
            e8rev.reshape(NT, P, KC // 2, 2, P).transpose(0, 1, 2, 4, 3)
        ).reshape(NT, P, KC // 2, 2 * P)
        m = {
            "enc": np.ascontiguousarray(ev),
            "encT": encT,
            "encT8": enc8i,
            "wb": wb,
            "w8": w8,
        }
        if with_bias:
            m["bb"] = bb
            m["b8"] = b8
        in_maps.append(m)
    return in_maps, with_bias


def kernel(hidden_dec, hidden_enc, W, b):
    in_maps, with_bias = make_in_maps(hidden_dec, hidden_enc, W, b)
    nc = build_program(with_bias)
    res = bass_utils.run_bass_kernel_spmd(nc, in_maps, core_ids=list(range(NCORES)))
    outp = np.concatenate([res.results[c]["out"] for c in range(NCORES)], axis=0)
    perm = _perm(np.asarray(hidden_dec, dtype=np.float32).reshape(D))
    out = np.empty_like(outp)
    out[:, perm] = outp
    return out.astype(np.float32)


# revision 49
# speedup vs baseline: 1.0733x; 1.0733x over previous
"""Trainium2 Bass kernel for nn_Attention_75814762709205.

Computation (per batch row b, seq s):
    proj  = hidden_enc @ W + b          [B,S,D]
    score = hidden_dec.T * proj         (per-channel scale)
    attn  = softmax(score, axis=-1)     (over D)
    out   = sum_s attn * hidden_enc     [B,D]

Sharding: data-parallel over batch, 4 batches per core on 8 cores.

Precision strategy (validated numerically against the exact reference
data): the softmax error from an fp8 matmul is amplified by exp() in
proportion to |dec_e|, so the channels are permuted by |dec| on the
host.  The top NB=256 channels go through a bf16 matmul; the remaining
NF=768 channels go through an fp8e4 DoubleRow matmul (2 k-chunks per
pass, ~1.8x the bf16 matmul rate).  dec is folded into W on the host
(score = enc @ (W*dec)), and the channel permutation is applied to both
W axes so a single permuted bf16 copy of enc serves as the matmul
contraction input and the attention values.  Softmax denominator is
folded into the final sequence reduction: context = sum_s r_s *
(exp_s * enc_s) with r = 1/sum(exp), computed as a matmul with r as the
stationary vector.

The transposed enc needed for the matmul stationaries is prepared on
the HOST (numpy reshape/transpose into the exact SBUF tile layout) and
DMA'd as a second stream, so the PE does no transposes at all; the fp8
copy of the transposed tile is converted on DVE.  The ctx matmuls run
two tiles behind so the PE never waits on the ACT/DVE softmax tail.
"""

import sys

sys.path.insert(0, "/opt/trn_rl_repo")

from collections import deque

import numpy as np
import ml_dtypes

import concourse.bass as bass
import concourse.mybir as mybir
import concourse.tile as tile
from concourse import bacc, bass_utils

B, S, D = 32, 2048, 1024
NCORES = 8
BPC = B // NCORES  # batches per core
ROWS = BPC * S  # rows per core
P = 128
NT = ROWS // P  # row tiles per core
TPB = S // P  # row tiles per batch
KC = D // P  # contraction chunks
NB = 256  # bf16 (high |dec|) channels
NF = D - NB  # fp8 channels
W8SCALE = 8.0  # fp8 W pre-scale (undone in the exp activation)

F32 = mybir.dt.float32
BF16 = mybir.dt.bfloat16
F8 = mybir.dt.float8e4
AF = mybir.ActivationFunctionType
DR = mybir.MatmulPerfMode.DoubleRow
DRS = mybir.MatmulPerfMode.DoubleRowSwInterleave

NP_BF16 = ml_dtypes.bfloat16
NP_F8 = ml_dtypes.float8_e4m3


def build_program(with_bias: bool, repeats: int = 1):
    nc = bacc.Bacc("TRN2", target_bir_lowering=False, debug=False)
    enc_d = nc.dram_tensor("enc", [ROWS, D], BF16, kind="ExternalInput")
    encT_d = nc.dram_tensor("encT", [NT, P, KC, P], BF16, kind="ExternalInput")
    encT8_d = nc.dram_tensor("encT8", [NT, P, KC // 2, 2 * P], F8, kind="ExternalInput")
    wb_d = nc.dram_tensor("wb", [P, KC, NB], BF16, kind="ExternalInput")
    w8_d = nc.dram_tensor("w8", [P, KC, NF], F8, kind="ExternalInput")
    bb_d = b8_d = None
    if with_bias:
        bb_d = nc.dram_tensor("bb", [1, NB], BF16, kind="ExternalInput")
        b8_d = nc.dram_tensor("b8", [1, NF], BF16, kind="ExternalInput")
    out_d = nc.dram_tensor("out", [BPC, D], F32, kind="ExternalOutput")

    NTOT = repeats * NT

    with tile.TileContext(nc) as tc:
        with (
            tc.tile_pool(name="consts", bufs=1) as consts,
            tc.tile_pool(name="wpool", bufs=1) as wpool,
            tc.tile_pool(name="encp", bufs=6) as encp,
            tc.tile_pool(name="encTp", bufs=5) as encTp,
            tc.tile_pool(name="encT8p", bufs=5) as encT8p,
            tc.tile_pool(name="expp", bufs=3) as expp,
            tc.tile_pool(name="wtp", bufs=4) as wtp,
            tc.tile_pool(name="smalls", bufs=12) as smalls,
            tc.tile_pool(name="outp", bufs=2) as outp,
            tc.tile_pool(name="pr_ps", bufs=3, space=bass.MemorySpace.PSUM) as pr_ps,
            tc.tile_pool(name="ctx_ps", bufs=1, space=bass.MemorySpace.PSUM) as ctx_ps,
        ):
            # ---- pipeline stages ----
            # three DMA streams per tile, issue spread over SP and ScalarE
            # queues (ScalarE is also a HWDGE engine) to avoid serializing on
            # one descriptor queue
            loaded = {}

            def load(t):
                enc_t = encp.tile([P, D], BF16, name="enc_t")
                nc.sync.dma_start(
                    enc_t, enc_d.ap()[(t % NT) * P : (t % NT + 1) * P, :]
                )
                encT = encTp.tile([P, KC, P], BF16, name="encT")
                nc.gpsimd.dma_start(encT, encT_d.ap()[t % NT])
                encT8 = encT8p.tile([P, KC // 2, 2 * P], F8, name="encT8")
                nc.scalar.dma_start(encT8, encT8_d.ap()[t % NT])
                loaded[t] = (enc_t, encT, encT8)

            state = {"ctx_half": None}

            def emit_ctx(prev):
                ones_col, wt, tib, bidx = prev
                if tib == 0:
                    state["ctx_half"] = [
                        ctx_ps.tile([1, 512], F32, name=f"ctxh{h}") for h in range(2)
                    ]
                for h2 in range(2):
                    nc.tensor.matmul(
                        state["ctx_half"][h2],
                        ones_col,
                        wt[:, h2 * 512 : (h2 + 1) * 512],
                        start=(tib == 0),
                        stop=(tib == TPB - 1),
                    )
                if tib == TPB - 1:
                    ctx_sb = outp.tile([1, D], F32, name="ctx_sb")
                    for h2 in range(2):
                        nc.vector.tensor_copy(
                            ctx_sb[:, h2 * 512 : (h2 + 1) * 512],
                            state["ctx_half"][h2],
                        )
                    nc.sync.dma_start(out_d.ap()[bidx : bidx + 1, :], ctx_sb)

            # bootstrap the enc pipeline BEFORE the W setup DMAs
            load(0)
            load(1)
            load(2)

            wb_sb = wpool.tile([P, KC, NB], BF16)
            nc.sync.dma_start(wb_sb, wb_d.ap())
            w8_sb = wpool.tile([P, KC, NF], F8)
            nc.sync.dma_start(w8_sb, w8_d.ap())

            # constant ones stationary for the ctx (sequence-sum) matmuls;
            # the softmax reciprocal is folded into wt on DVE instead
            ones_f = consts.tile([P, 1], F32)
            nc.any.memset(ones_f, 1.0)
            ones_col = consts.tile([P, 1], BF16)
            nc.vector.tensor_copy(ones_col, ones_f)

            ones_row = None
            bb_sb = b8_sb = None
            if with_bias:
                ones_f32 = consts.tile([1, P], F32)
                nc.any.memset(ones_f32, 1.0)
                ones_row = consts.tile([1, P], BF16)
                nc.vector.tensor_copy(ones_row, ones_f32)
                bb_sb = consts.tile([1, NB], BF16)
                nc.sync.dma_start(bb_sb, bb_d.ap())
                b8_sb = consts.tile([1, NF], BF16)
                nc.sync.dma_start(b8_sb, b8_d.ap())

            pending = deque()
            for t in range(NTOT):
                bidx, tib = divmod(t % NT, TPB)
                if t + 3 < NTOT:
                    load(t + 3)
                enc_t, encT, encT8 = loaded.pop(t)

                # score layout (permuted channels): cols 0:NB bf16 part,
                # NB:1024 fp8 part.  One 2-bank PSUM tile; bank0 holds
                # [bf16 256 | fp8 256], bank1 holds fp8 512.
                pr = pr_ps.tile([P, 1024], F32, name="pr")

                # bf16 part: shares bank0 with the first fp8 region
                # (disjoint columns; the fp8 matmuls use start=False and
                # overwrite-on-first-touch via has_written)
                for k in range(KC):
                    nc.tensor.matmul(
                        pr[:, 0:NB],
                        encT[:, k, :],
                        wb_sb[:, k, :],
                        start=(k == 0),
                        stop=False,
                        skip_group_check=True,
                    )
                # fp8 DoubleRow part (software-interleaved stationary: the
                # host lays each k-pair out contiguously, so LDWEIGHTS is a
                # linear read instead of the HW interleave pattern)
                for c in range(KC // 2):
                    if c == 2 and len(pending) >= 2:
                        emit_ctx(pending.popleft())
                    ks = slice(2 * c, 2 * c + 2)
                    last = c == KC // 2 - 1
                    nc.tensor.matmul(
                        pr[:, NB:512],
                        encT8[:, c, :],
                        w8_sb[:, ks, 0 : 512 - NB],
                        start=False,
                        stop=(last and not with_bias),
                        perf_mode=DRS,
                        skip_group_check=True,
                    )
                    nc.tensor.matmul(
                        pr[:, 512:1024],
                        encT8[:, c, :],
                        w8_sb[:, ks, 512 - NB : NF],
                        start=(c == 0),
                        stop=(last and not with_bias),
                        perf_mode=DRS,
                        skip_group_check=True,
                    )
                if with_bias:
                    nc.tensor.matmul(
                        pr[:, 0:NB],
                        ones_row,
                        bb_sb,
                        start=False,
                        stop=False,
                        skip_group_check=True,
                    )
                    nc.tensor.matmul(
                        pr[:, NB:512],
                        ones_row,
                        b8_sb[:, 0 : 512 - NB],
                        start=False,
                        stop=True,
                        skip_group_check=True,
                    )
                    nc.tensor.matmul(
                        pr[:, 512:1024],
                        ones_row,
                        b8_sb[:, 512 - NB : NF],
                        start=False,
                        stop=True,
                        skip_group_check=True,
                    )

                # single exp over both banks with fused row-sum; the whole W
                # carries the W8SCALE pre-scale (exact power of 2), undone here
                ssum = smalls.tile([P, 1], F32)
                exp_t = expp.tile([P, D], BF16)
                nc.scalar.activation(
                    exp_t, pr, AF.Exp, scale=1.0 / W8SCALE, accum_out=ssum
                )

                recip_f = smalls.tile([P, 1], F32)
                nc.vector.reciprocal(recip_f, ssum)

                # wt = (exp * 1/rowsum) * enc fused on DVE; ctx then reduces
                # over rows with a constant ones stationary
                wt = wtp.tile([P, D], BF16)
                nc.vector.scalar_tensor_tensor(
                    wt, exp_t, recip_f, enc_t,
                    op0=mybir.AluOpType.mult, op1=mybir.AluOpType.mult,
                )

                pending.append((ones_col, wt, tib, bidx))
            while pending:
                emit_ctx(pending.popleft())

    nc.compile()
    return nc


def _perm(dec):
    return np.argsort(-np.abs(dec), kind="stable")


def make_in_maps(hidden_dec, hidden_enc, W, b):
    enc = np.asarray(hidden_enc, dtype=np.float32).reshape(B, S, D)
    W = np.asarray(W, dtype=np.float32)
    dec = np.asarray(hidden_dec, dtype=np.float32).reshape(D)
    b = np.asarray(b, dtype=np.float32).reshape(D)
    with_bias = bool(np.any(b != 0.0))

    perm = _perm(dec)
    Weff = W * dec[None, :]
    Wp = Weff[np.ix_(perm, perm)]
    wb = np.ascontiguousarray(
        (Wp[:, :NB] * W8SCALE).reshape(KC, P, NB).transpose(1, 0, 2)
    ).astype(NP_BF16)
    w8 = np.ascontiguousarray(
        (Wp[:, NB:] * W8SCALE).reshape(KC, P, NF).transpose(1, 0, 2)
    ).astype(NP_F8)
    encp = enc[:, :, perm].astype(NP_BF16)

    b_eff = (b * dec)[perm] * W8SCALE
    bb = b_eff[:NB].reshape(1, NB).astype(NP_BF16)
    b8 = b_eff[NB:].reshape(1, NF).astype(NP_BF16)

    in_maps = []
    for c in range(NCORES):
        ev = encp[c * BPC : (c + 1) * BPC].reshape(ROWS, D)
        # host-side tiled transpose into the exact SBUF stationary layout:
        # encT[t, p, kc, r] = enc[t*128 + r, kc*128 + p]
        encT = np.ascontiguousarray(
            ev.reshape(NT, P, KC, P).transpose(0, 3, 2, 1)
        )
        # fp8 copy, software-interleaved for DoubleRowSwInterleave: per
        # partition each k-pair's stationary stream is
        # [A_col127, B_col127, A_col126, ..., B_col0] (A/B = the two
        # k-chunks, columns reversed)
        e8rev = encT.astype(NP_F8)[:, :, :, ::-1]
        enc8i = np.ascontiguousarray(
            e8rev.reshape(NT, P, KC // 2, 2, P).transpose(0, 1, 2, 4, 3)
        ).reshape(NT, P, KC // 2, 2 * P)
        m = {
            "enc": np.ascontiguousarray(ev),
            "encT": encT,
            "encT8": enc8i,
            "wb": wb,
            "w8": w8,
        }
        if with_bias:
            m["bb"] = bb
            m["b8"] = b8
        in_maps.append(m)
    return in_maps, with_bias


def kernel(hidden_dec, hidden_enc, W, b):
    in_maps, with_bias = make_in_maps(hidden_dec, hidden_enc, W, b)
    nc = build_program(with_bias)
    res = bass_utils.run_bass_kernel_spmd(nc, in_maps, core_ids=list(range(NCORES)))
    outp = np.concatenate([res.results[c]["out"] for c in range(NCORES)], axis=0)
    perm = _perm(np.asarray(hidden_dec, dtype=np.float32).reshape(D))
    out = np.empty_like(outp)
    out[:, perm] = outp
    return out.astype(np.float32)
